# revision 19
# baseline (speedup 1.0000x reference)
"""Deformable Conv2D (DCNv2-style) on 8 Trainium2 NeuronCores.

Strategy (data-parallel over batch, one sample per core):
  conv-first reformulation:  out[f,j] = sum_kk sum_corner w_corner[kk,j] * Y_kk[f, p_corner(kk,j)]
  where Y_kk = W[:,:,kk] @ x  (plain matmul over all spatial positions).

  Sampling uses a per-triplet DRAM table TC2[g] ([3, rows, 256] bf16) whose
  row r packs the 2-slot corner pair [Y(r-65) | Y(r-1)].  One dma_gather
  descriptor per (tap, output position) fetches rows t' and t'+1
  (elem_size=512 elems, elem_step=256), i.e. all 4 bilinear corners
  [Y(p00)|Y(p10)|Y(p01)|Y(p11)].  The Q7 descriptor generator costs ~6us
  fixed + ~5ns/idx, so taps 0-7 use one 4096-idx gather each; tap 8 is
  split in 1024-idx batches so transposes/stores pipeline into the tail.

  Because slot1 of row r equals slot0 of row r+64, the table is produced by a
  SINGLE shifted matmul pass (Y^T tiles) whose staging tile is DMA'd twice
  (once per slot at different row offsets) -- no 4x matmul redundancy.

  Bilinear/mask/validity weights are folded into 4 per-position corner
  weights on the host and applied as fat DVE tensor_tensor ops (stride-0
  broadcast along f).

Shapes (hardcoded per spec): x (8,128,64,64) f32, offset (8,18,64,64),
mask (8,9,64,64), weight (128,128,3,3), out (8,128,64,64) f32.
"""

import numpy as np
import ml_dtypes
from contextlib import ExitStack

import concourse.bass as bass
import concourse.bacc as bacc
import concourse.tile as tile
from concourse import mybir
from concourse.bass_utils import run_bass_kernel_spmd

B, C, H, W = 8, 128, 64, 64
F = 128
KH = KW = 3
KK = KH * KW
HW = H * W  # 4096
NP = 128
NJB = HW // NP  # 32 j-blocks
NTT = 34  # Y^T pass tiles; rows r in [0, 4352)
TROWS = NTT * NP  # 4352
RPAD = 64  # head pad rows (slot1 writes reach row -64)
TCOLS = 256  # 2 slots x 128 f: [Y(r-65) | Y(r-1)]
TBL_ROWS = RPAD + TROWS  # 4416
TBL_KK = TBL_ROWS * TCOLS
XPAD = TROWS  # x_sb col q holds x[q - 65] (zeros outside)
XOFF = 65
REP = 16  # weight repeat factor

SINGLE_PACKET = False

BF16 = mybir.dt.bfloat16
F32 = mybir.dt.float32
I16 = mybir.dt.int16


def _prep_indices_weights(offset, mask):
    """Per-sample host prep. offset [18,H,W], mask [9,H,W] ->
    idx int16 [128, KK*256], wts bf16 [128, KK*4*NJB] (corner order
    c00, c10, c01, c11 to match the 2-slot gathered row layout)."""
    off = offset.reshape(KK, 2, H, W)
    dy, dx = off[:, 0], off[:, 1]
    ki, kj = np.meshgrid(np.arange(KH), np.arange(KW), indexing="ij")
    ki = ki.reshape(KK, 1, 1).astype(np.float32)
    kj = kj.reshape(KK, 1, 1).astype(np.float32)
    base_y = (np.arange(H, dtype=np.float32) - 1.0)[None, :, None] + ki
    base_x = (np.arange(W, dtype=np.float32) - 1.0)[None, None, :] + kj
    py = base_y + dy
    px = base_x + dx
    y0 = np.floor(py)
    x0 = np.floor(px)
    ly = (py - y0).astype(np.float32)
    lx = (px - x0).astype(np.float32)
    hy = 1.0 - ly
    hx = 1.0 - lx
    y0i = y0.astype(np.int64)
    x0i = x0.astype(np.int64)

    vy0 = (y0i >= 0) & (y0i < H)
    vy1 = (y0i + 1 >= 0) & (y0i + 1 < H)
    vx0 = (x0i >= 0) & (x0i < W)
    vx1 = (x0i + 1 >= 0) & (x0i + 1 < W)

    m = mask.reshape(KK, H, W)
    w00 = (hy * hx * m * (vy0 & vx0)).reshape(KK, HW).astype(np.float32)
    w01 = (hy * lx * m * (vy0 & vx1)).reshape(KK, HW).astype(np.float32)
    w10 = (ly * hx * m * (vy1 & vx0)).reshape(KK, HW).astype(np.float32)
    w11 = (ly * lx * m * (vy1 & vx1)).reshape(KK, HW).astype(np.float32)

    flat = np.clip(y0i * W + x0i + 65, 0, HW + 64).reshape(KK, HW)

    # idx: per kk, 4096 ordinals j wrapped o -> [o%16, o//16], replicated to
    # 128 partitions (dma_gather consumes idxs from each 16-partition group).
    idx_dev = np.empty((128, KK * 256), np.int16)
    for kk in range(KK):
        wrapped = flat[kk].astype(np.int16).reshape(256, 16).T  # [16, 256]
        idx_dev[:, kk * 256 : (kk + 1) * 256] = np.tile(wrapped, (8, 1))

    # wts: [128, (kk, corner, i, rep)]; value[p] = w_c[kk, i*128+p], each
    # weight repeated REP times (16-wide runs let DVE mults hit 2x mode).
    # corner order matches gathered elem: [Y(p00)|Y(p10)|Y(p01)|Y(p11)]
    corners = (w00, w10, w01, w11)
    w4 = np.stack([c.reshape(KK, NJB, 128) for c in corners], axis=1)
    w4 = np.repeat(w4[..., None], REP, axis=4)  # [KK,4,NJB,128,REP]
    wts_dev = np.ascontiguousarray(
        np.transpose(w4, (3, 0, 1, 2, 4)).reshape(128, -1)
    ).astype(ml_dtypes.bfloat16)
    return idx_dev, wts_dev


def _split_overfull_waits(nc):
    """This walrus build accepts 1 sync-wait per instruction (2 for EVSEM).
    Move extras onto preceding same-engine NoOps."""
    for f in nc.m.functions:
        for bb in f.blocks:
            new_list = []
            for ins in bb.instructions:
                si = ins.sync_info
                waits = list(si.on_wait) if si and si.on_wait else []
                cap = 2 if isinstance(ins, mybir.InstEventSemaphore) else 1
                if len(waits) > cap:
                    extra, keep = waits[:-cap], waits[-cap:]
                    for k, w in enumerate(extra):
                        nop = mybir.InstNoOp(
                            name=f"{ins.name}_waitsplit{k}",
                            sync_info=mybir.SyncInfo(on_wait=[w], on_update=[]),
                            bass_nofuse=True,
                            engine=ins.engine,
                        )
                        new_list.append(nop)
                        nc.register_instruction(nop, overwrite=True)
                    si.on_wait = keep
                new_list.append(ins)
            bb.instructions[:] = new_list


def _revec(ap, dims):
    """Rebuild an AP keeping its partition dim, replacing free dims."""
    return bass.AP(ap.tensor, ap.offset, [list(ap.ap[0])] + [list(d) for d in dims])


def _build_nc():
    nc = bacc.Bacc(None, target_bir_lowering=False, debug=False)
    x_d = nc.dram_tensor("x", [NP, XPAD], BF16, kind="ExternalInput")
    wt_d = nc.dram_tensor("wt", [NP, KK * F], BF16, kind="ExternalInput")
    idx_d = nc.dram_tensor("idx", [NP, KK * 256], I16, kind="ExternalInput")
    wts_d = nc.dram_tensor("wts", [NP, KK * 4 * NJB * REP], BF16, kind="ExternalInput")
    ident_d = nc.dram_tensor("ident", [NP, NP], F32, kind="ExternalInput")
    out_d = nc.dram_tensor("out", [NP, HW], F32, kind="ExternalOutput")
    # one table tensor per kk-triplet: [3 kk, TBL_ROWS, TCOLS]
    tbl_d = [
        nc.dram_tensor(f"tbl{g}", [3, TBL_ROWS, TCOLS], BF16, kind="Internal")
        for g in range(3)
    ]

    with tile.TileContext(nc) as tc, ExitStack() as ctx:
        cpool = ctx.enter_context(tc.tile_pool(name="const", bufs=1))
        tcst_pool = ctx.enter_context(tc.tile_pool(name="tcst", bufs=10))
        gpool = ctx.enter_context(tc.tile_pool(name="gat", bufs=3))
        g8pool = ctx.enter_context(tc.tile_pool(name="gat8", bufs=3))
        tpool = ctx.enter_context(tc.tile_pool(name="tmp", bufs=1))
        accpool = ctx.enter_context(tc.tile_pool(name="acc", bufs=1))
        pspool = ctx.enter_context(tc.tile_pool(name="ps", bufs=4, space="PSUM"))
        ptpool = ctx.enter_context(tc.tile_pool(name="pst", bufs=2, space="PSUM"))

        x_sb = cpool.tile([NP, XPAD], BF16)
        wt_sb = cpool.tile([NP, KK * F], BF16)
        idx_sb = cpool.tile([NP, KK * 256], I16)
        wts_sb = cpool.tile([NP, KK * 4 * NJB * REP], BF16)
        id_sb = cpool.tile([NP, NP], F32)
        acc_sb = accpool.tile([NP, HW], F32)
        out_sb = accpool.tile([NP, HW], F32)

        nc.sync.dma_start(x_sb[:], x_d[:])
        nc.scalar.dma_start(wt_sb[:], wt_d[:])
        nc.scalar.dma_start(idx_sb[:], idx_d[:])

        # ---- Stage A: build 2-slot tables, one shifted-matmul pass.
        # Y^T tile rows r = tt*128+q hold Y(r-65)[f] for 3 kk (N=384).
        # Written twice: slot0 at rows r, slot1 at rows r-64.
        def build_tables(g):
            for tt in range(NTT):
                ps = pspool.tile([NP, 512], F32)
                tcst = tcst_pool.tile([NP, 3, F], BF16)
                nc.tensor.matmul(
                    ps[:, 0 : 3 * F],
                    x_sb[:, tt * NP : (tt + 1) * NP],
                    wt_sb[:, g * 3 * F : (g + 1) * 3 * F],
                    start=True,
                    stop=True,
                )
                nc.scalar.copy(tcst[:], ps[:, 0 : 3 * F].rearrange("p (k f) -> p k f", k=3))
                dstA = bass.AP(
                    tbl_d[g],
                    (RPAD + tt * NP) * TCOLS,
                    [[TCOLS, NP], [TBL_KK, 3], [1, F]],
                )
                nc.sync.dma_start(dstA, tcst[:])
                dstB = bass.AP(
                    tbl_d[g],
                    tt * NP * TCOLS + F,
                    [[TCOLS, NP], [TBL_KK, 3], [1, F]],
                )
                nc.scalar.dma_start(dstB, tcst[:])

        # ---- Stage B: gather + weighted accumulate for i-blocks [i0, i0+ni)
        def gather_combine(kk, i0, ni, pool):
            g_t = pool.tile([NP, ni, 512], BF16, tag="g_t")
            src = bass.AP(
                tbl_d[kk // 3],
                (kk % 3) * TBL_KK + RPAD * TCOLS,
                [[TCOLS, HW + 66], [1, 512]],
            )
            nc.gpsimd.dma_gather(
                out_ap=g_t[:],
                in_ap=src,
                idxs_ap=idx_sb[:, kk * 256 + i0 * 8 : kk * 256 + (i0 + ni) * 8],
                num_idxs=NP * ni,
                num_idxs_reg=NP * ni,
                elem_size=512,
                elem_step=TCOLS,
                single_packet=SINGLE_PACKET,
            )
            t0 = tpool.tile([NP, ni, 8, REP], BF16, tag="t0")
            t1 = tpool.tile([NP, ni, 8, REP], BF16, tag="t1")
            t2 = tpool.tile([NP, ni, 8, REP], BF16, tag="t2")
            t3 = tpool.tile([NP, ni, 8, REP], BF16, tag="t3")
            mu = mybir.AluOpType.mult
            ad = mybir.AluOpType.add
            gap = g_t[:]

            def g_c(c):
                return bass.AP(
                    gap.tensor, gap.offset + c * F,
                    [list(gap.ap[0]), [512, ni], [REP, 8], [1, REP]],
                )

            def w_c(c):
                base = ((kk * 4 + c) * NJB + i0) * REP
                sl = wts_sb[:, base : base + ni * REP]
                return _revec(sl, [[REP, ni], [0, 8], [1, REP]])

            nc.vector.tensor_tensor(t0[:], g_c(0), w_c(0), mu)
            nc.vector.tensor_tensor(t1[:], g_c(1), w_c(1), mu)
            nc.vector.tensor_tensor(t2[:], g_c(2), w_c(2), mu)
            nc.vector.tensor_tensor(t3[:], g_c(3), w_c(3), mu)
            nc.vector.tensor_tensor(t0[:], t0[:], t1[:], ad)
            nc.vector.tensor_tensor(t2[:], t2[:], t3[:], ad)
            a_sl = acc_sb[:, i0 * NP : (i0 + ni) * NP].rearrange(
                "p (i f) -> p i f", i=ni
            )
            t0v = _revec(t0[:], [[F, ni], [1, F]])
            t2v = _revec(t2[:], [[F, ni], [1, F]])
            if kk == 0:
                nc.vector.tensor_tensor(a_sl, t0v, t2v, ad)
            else:
                nc.vector.tensor_tensor(t0[:], t0[:], t2[:], ad)
                nc.vector.tensor_tensor(a_sl, a_sl, t0v, ad)

        # ---- Stage C: transpose acc [p, f] tiles -> out [f, j]
        def transpose_out(i0, ni):
            for jb in range(i0, i0 + ni):
                pst = ptpool.tile([NP, NP], F32, tag="pst")
                nc.tensor.transpose(pst[:], acc_sb[:, jb * NP : (jb + 1) * NP], id_sb[:])
                nc.scalar.copy(out_sb[:, jb * NP : (jb + 1) * NP], pst[:])
            nc.sync.dma_start(
                out_d[:, i0 * NP : (i0 + ni) * NP], out_sb[:, i0 * NP : (i0 + ni) * NP]
            )

        build_tables(0)
        nc.scalar.dma_start(wts_sb[:], wts_d[:])
        nc.scalar.dma_start(id_sb[:], ident_d[:])
        build_tables(1)
        build_tables(2)
        for kk in range(KK):
            if kk < KK - 1:
                gather_combine(kk, 0, 16, gpool)
                gather_combine(kk, 16, 16, gpool)
            else:
                for q in range(4):
                    gather_combine(kk, q * 8, 8, g8pool)
                    transpose_out(q * 8, 8)

    nc.compile()
    _split_overfull_waits(nc)
    return nc


_NC_CACHE = {}


def _get_nc():
    if "nc" not in _NC_CACHE:
        _NC_CACHE["nc"] = _build_nc()
    return _NC_CACHE["nc"]


def _prep_x(xb):
    """x [C,H,W] f32 -> padded bf16 [128, XPAD]; col q = x[q - 65]."""
    xp = np.zeros((C, XPAD), ml_dtypes.bfloat16)
    xp[:, XOFF : XOFF + HW] = xb.reshape(C, HW).astype(ml_dtypes.bfloat16)
    return xp


def kernel(x, offset, mask, weight, **run_kwargs):
    x = np.asarray(x, np.float32)
    offset = np.asarray(offset, np.float32)
    mask = np.asarray(mask, np.float32)
    weight = np.asarray(weight, np.float32)

    wt = np.transpose(weight.reshape(F, C, KK), (1, 2, 0)).reshape(C, KK * F)
    wt = np.ascontiguousarray(wt).astype(ml_dtypes.bfloat16)
    ident = np.eye(NP, dtype=np.float32)

    in_maps = []
    for b in range(B):
        idx_dev, wts_dev = _prep_indices_weights(offset[b], mask[b])
        in_maps.append(
            {
                "x": _prep_x(x[b]),
                "wt": wt,
                "idx": idx_dev,
                "wts": wts_dev,
                "ident": ident,
            }
        )

    nc = _get_nc()
    res = run_bass_kernel_spmd(nc, in_maps, core_ids=list(range(8)), **run_kwargs)
    out = np.stack([np.asarray(res.results[b]["out"]).reshape(F, H, W) for b in range(B)])
    if run_kwargs:
        kernel.last_results = res
    return out
